# revision 21
# baseline (speedup 1.0000x reference)
"""Trainium2 Bass kernel for nn_Fine_Change_Moment3.

Math (from the reference):
  - input (16,512,512,16) [b,y,x,t]; fc_weight3 (262144,16,6) per-patch 16x6.
  - Only channel 0 of the CAM survives (cam[:, 0]), so only
    fc_weight3[:, :, 0] matters (host-sliced).
  - Per 4x4 patch n=(b,gy,gx): cam0[t] = sum_k patch[k,t] * w[n,k]
  - v = (cam0 - min_t) / max_t(cam0 - min_t)
  - top[b,t] = v arranged (gy,gx); up = A @ top @ A^T with A the 128->512
    bilinear (align_corners) interp matrix; output (b*512*512, 16) f32.

Distribution: data-parallel over batch, 2 batches per core, 8 cores.

fp16 end-to-end on device: the host casts input and weights to fp16 and
casts the fp16 output back to f32, halving HBM traffic (the DMA roofline)
and doubling DVE/PE throughput (2e-2 tolerance; fp16 leaves ~1e-2 margin).

Engine queues are FIFO, so program order is the schedule. The two batches
are software-pipelined explicitly:

  A: load+weight+reduce (s123) b0          DVE-paced, in_b0 streams
  B: norm+transpose+M1 b0; prefetch in_b1
  C: up/out b0 INTERLEAVED with s123 b1    ACT+PE run b0's tail while
                                           DVE consumes b1's input;
                                           out_b0 + in_b1 share DMA
  D: norm+transpose+M1 b1
  E: up/out b1                             copies split DVE/ACT (nothing
                                           queues behind them)

Stage detail:
  s123: DVE multiply by per-patch weights (weights host-duplicated x2 so
        the t-broadcast keeps a dense step-1 inner pair -> 2x DVE mode),
        DVE pairwise add tree over px, PE 0/1-selection matmul reduces py
        and regroups gy -> cam[gy][(gx,t)] accumulated fp32 in PSUM.
  norm: DVE min/max over t; min/recip pair-duplicated (gpsimd) so the
        broadcast subtract/multiply stay in 2x mode.
  M1:   PE per-t transposes then M1_t = topT_t^T @ A^T, ACT interleaves
        pairs into m1i[gy][(sx,t)].
  up:   PE (A^T chunk)^T @ m1i chunk -> PSUM, copies cast to fp16 staging,
        contiguous DMA out.
"""

import numpy as np

B, S, T, PP = 16, 512, 16, 4
G = S // PP          # 128 patch grid
NCORES = 8
BPC = B // NCORES    # 2 batches per core

_CACHE = {}


def _interp_matrix_np(n_in, n_out):
    # mirrors the reference's align_corners=True bilinear matrix
    coords = np.arange(n_out, dtype=np.float32) * ((n_in - 1) / (n_out - 1))
    i0 = np.clip(np.floor(coords).astype(np.int64), 0, n_in - 2)
    w = coords - i0.astype(np.float32)
    A = np.zeros((n_out, n_in), dtype=np.float32)
    rows = np.arange(n_out)
    np.add.at(A, (rows, i0), 1.0 - w)
    np.add.at(A, (rows, i0 + 1), w)
    return A  # (n_out, n_in)


def _build_program():
    from contextlib import ExitStack
    import concourse.bacc as bacc
    import concourse.tile as tile
    import concourse.mybir as mybir

    f32 = mybir.dt.float32
    f16 = mybir.dt.float16
    Alu = mybir.AluOpType
    Ax = mybir.AxisListType

    nc = bacc.Bacc("TRN2", target_bir_lowering=False, debug=False,
                   num_devices=NCORES)

    x_d = nc.dram_tensor("x", [BPC, S, S, T], f16, kind="ExternalInput")
    w_d = nc.dram_tensor("w", [BPC, 128, 4096], f16, kind="ExternalInput")
    at_d = nc.dram_tensor("at", [128, 512], f16, kind="ExternalInput")
    sel_d = nc.dram_tensor("sel", [128, 512], f16, kind="ExternalInput")
    id_d = nc.dram_tensor("ident", [128, 128], f16, kind="ExternalInput")
    y_d = nc.dram_tensor("y", [BPC, S, S, T], f16, kind="ExternalOutput")

    # input view: [b][yt][xh][y_row=128][(x256 t16)=4096]
    x_v = x_d.ap().rearrange("b (yt p) (xh xx) t -> b yt xh p (xx t)",
                             p=128, xh=2)
    # output view: [b][syc][xh][sy=128][(xx256 t16)=4096]
    y_v = y_d.ap().rearrange("b (syc sy) (xh xx) t -> b syc xh sy (xx t)",
                             syc=4, xh=2)

    with tile.TileContext(nc) as tc, ExitStack() as ctx:
        consts = ctx.enter_context(tc.tile_pool(name="consts", bufs=1))
        pin = ctx.enter_context(tc.tile_pool(name="pin", bufs=4))
        pw = ctx.enter_context(tc.tile_pool(name="pw", bufs=2))
        pp2 = ctx.enter_context(tc.tile_pool(name="pp2", bufs=4))
        pcam = ctx.enter_context(tc.tile_pool(name="pcam", bufs=1))
        pv = ctx.enter_context(tc.tile_pool(name="pv", bufs=1))
        ptop = ctx.enter_context(tc.tile_pool(name="ptop", bufs=1))
        pm1 = ctx.enter_context(tc.tile_pool(name="pm1", bufs=1))
        pst = ctx.enter_context(tc.tile_pool(name="pst", bufs=3))
        ps1 = ctx.enter_context(tc.tile_pool(name="ps1", bufs=2))
        # PSUM: cam+transpose share a 4-bank pool; m1/up share a
        # 2x 2-bank pool -> static total 8 banks.
        ppsc = ctx.enter_context(tc.tile_pool(name="ppsc", bufs=1,
                                              space="PSUM"))
        ppsmu = ctx.enter_context(tc.tile_pool(name="ppsmu", bufs=2,
                                               space="PSUM"))

        at_sb = consts.tile([128, 512], f16)
        sel_sb = consts.tile([128, 512], f16)
        id_sb = consts.tile([128, 128], f16)

        state = {}

        def s123_open(b):
            # weight prefetch only; cam PSUM is allocated lazily at its
            # first matmul so the ppsc ring order stays cam(b)/tp(b)
            state[b] = {
                "cam_ps": None,
                "w_sb": pw.tile([128, 4096], f16, tag="w", name="w_sb"),
                "p2": None,
            }
            nc.sync.dma_start(state[b]["w_sb"][:], w_d.ap()[b])

        def s123_unit(b, u):
            # one (yt, xh) input unit: DMA + multiply + add tree (+ cam
            # matmuls when the yt pair completes)
            yt, xh = u // 2, u % 2
            st = state[b]
            w_sb = st["w_sb"]
            if xh == 0:
                st["p2"] = pp2.tile([128, 2048], f16, tag="p2", name="p2")
            p2 = st["p2"]
            it = pin.tile([128, 4096], f16, tag="in")
            nc.sync.dma_start(it[:], x_v[b, yt, xh])
            itv = it[:].rearrange("p (x tp two) -> p x tp two",
                                  tp=T // 2, two=2)
            wv = (w_sb[:, yt * 1024 + xh * 512:yt * 1024 + (xh + 1) * 512]
                  .rearrange("p (x two) -> p x two", two=2)
                  .unsqueeze(2)
                  .broadcast_to([128, 256, T // 2, 2]))
            nc.vector.tensor_tensor(itv, itv, wv, op=Alu.mult)
            pr = it[:].rearrange("p (gx pxp px2 t) -> p gx pxp px2 t",
                                 pxp=2, px2=2, t=T)
            s1 = ps1.tile([128, 2048], f16, tag="s1")
            s1v = s1[:].rearrange("p (gx pxp t) -> p gx pxp t",
                                  pxp=2, t=T)
            nc.vector.tensor_tensor(s1v, pr[:, :, :, 0, :],
                                    pr[:, :, :, 1, :], op=Alu.add)
            rout = (p2[:, xh * 1024:(xh + 1) * 1024]
                    .rearrange("p (gx t) -> p gx t", t=T))
            nc.vector.tensor_tensor(rout, s1v[:, :, 0, :],
                                    s1v[:, :, 1, :], op=Alu.add)
            if xh == 1:
                if st["cam_ps"] is None:
                    st["cam_ps"] = ppsc.tile([128, 2048], f32, tag="cam",
                                         name="cam_ps")
                cam_ps = st["cam_ps"]
                for fc in range(4):
                    nc.tensor.matmul(
                        cam_ps[:, fc * 512:(fc + 1) * 512],
                        lhsT=sel_sb[:, yt * 128:(yt + 1) * 128],
                        rhs=p2[:, fc * 512:(fc + 1) * 512],
                        start=(yt == 0), stop=(yt == 3),
                    )

        def norm_m1(b, help_dve):
            st = state[b]
            cam_ps = st["cam_ps"]
            cam = pcam.tile([128, 2048], f16, tag="cam")
            nc.scalar.copy(cam[:], cam_ps[:])

            v = pv.tile([128, 2048], f16, tag="v")
            mn = pv.tile([128, 128], f16, tag="mn")
            mnp = pv.tile([128, 256], f16, tag="mnp")
            mx = pv.tile([128, 128], f32, tag="mx")
            rx = pv.tile([128, 128], f32, tag="rx")
            rxp = pv.tile([128, 256], f16, tag="rxp")
            cam3 = cam[:].rearrange("p (gx t) -> p gx t", t=T)
            cam4 = cam[:].rearrange("p (gx tp two) -> p gx tp two",
                                    tp=T // 2, two=2)
            v3 = v[:].rearrange("p (gx t) -> p gx t", t=T)
            v4 = v[:].rearrange("p (gx tp two) -> p gx tp two",
                                tp=T // 2, two=2)
            nc.vector.tensor_reduce(mn[:], cam3, axis=Ax.X, op=Alu.min)
            nc.gpsimd.tensor_copy(
                mnp[:].rearrange("p (gx two) -> p gx two", two=2),
                mn[:].unsqueeze(2).broadcast_to([128, 128, 2]))
            mnb = (mnp[:].rearrange("p (gx two) -> p gx two", two=2)
                   .unsqueeze(2).broadcast_to([128, 128, T // 2, 2]))
            nc.vector.tensor_tensor(v4, cam4, mnb, op=Alu.subtract)
            nc.vector.tensor_reduce(mx[:], v3, axis=Ax.X, op=Alu.max)
            nc.vector.reciprocal(rx[:], mx[:])
            nc.gpsimd.tensor_copy(
                rxp[:].rearrange("p (gx two) -> p gx two", two=2),
                rx[:].unsqueeze(2).broadcast_to([128, 128, 2]))
            rxb = (rxp[:].rearrange("p (gx two) -> p gx two", two=2)
                   .unsqueeze(2).broadcast_to([128, 128, T // 2, 2]))
            nc.vector.tensor_tensor(v4, v4, rxb, op=Alu.mult)

            # per-t 128x128 transposes -> topT[gx][(t,gy)]
            topT = ptop.tile([128, 2048], f16, tag="top")
            vt = v[:].rearrange("p (gx t) -> p t gx", t=T)
            tp_ps = ppsc.tile([128, 2048], f16, tag="cam")
            for t in range(T):
                nc.tensor.transpose(tp_ps[:, t * 128:(t + 1) * 128],
                                    vt[:, t, :], id_sb[:])
            nc.scalar.copy(topT[:], tp_ps[:])

            # M1 per t -> m1i[gy][(sx,t)] fp16, pair-interleaved
            m1i = pm1.tile([128, 8192], f16, tag="m1i")
            m1iv = m1i[:].rearrange("p (sx t) -> p sx t", t=T)
            for tq in range(8):
                m1_ps = ppsmu.tile([128, 1024], f32, tag="mu")
                for tl in range(2):
                    t = tq * 2 + tl
                    nc.tensor.matmul(
                        m1_ps[:, tl * 512:(tl + 1) * 512],
                        lhsT=topT[:, t * 128:(t + 1) * 128],
                        rhs=at_sb[:],
                        start=True, stop=True,
                    )
                csrc = m1_ps[:].rearrange("p (tl sx) -> p sx tl", tl=2)
                cdst = m1iv[:, :, tq * 2:(tq + 1) * 2]
                if help_dve and tq % 2 == 1:
                    nc.vector.tensor_copy(cdst, csrc)
                else:
                    nc.scalar.copy(cdst, csrc)
            st["m1i"] = m1i

        def up_unit(b, u, help_dve):
            # one (syc, xh) output unit: 4x (2 matmuls + copy), DMA out
            syc, xh = u // 2, u % 2
            m1i = state[b]["m1i"]
            stg = pst.tile([128, 4096], f16, tag="stg")
            for sxg in range(4):
                up_ps = ppsmu.tile([128, 1024], f32, tag="mu")
                for sxl in range(2):
                    sxblk = (xh * 4 + sxg) * 2 + sxl
                    nc.tensor.matmul(
                        up_ps[:, sxl * 512:(sxl + 1) * 512],
                        lhsT=at_sb[:, syc * 128:(syc + 1) * 128],
                        rhs=m1i[:, sxblk * 512:(sxblk + 1) * 512],
                        start=True, stop=True,
                    )
                dst = stg[:, sxg * 1024:(sxg + 1) * 1024]
                if help_dve and sxg % 2 == 1:
                    nc.vector.tensor_copy(dst, up_ps[:])
                else:
                    nc.scalar.copy(dst, up_ps[:])
            nc.sync.dma_start(y_v[b, syc, xh], stg[:])

        # ---- phase A: s123(b0); consts ride behind the first input DMAs
        s123_open(0)
        s123_unit(0, 0)
        nc.sync.dma_start(at_sb[:], at_d.ap())
        nc.sync.dma_start(sel_sb[:], sel_d.ap())
        nc.sync.dma_start(id_sb[:], id_d.ap())
        for u in range(1, 8):
            s123_unit(0, u)

        # ---- phase B: norm+transpose+M1 b0; prefetch b1's first inputs
        s123_open(1)
        norm_m1(0, help_dve=False)

        # ---- phase C: up/out b0 interleaved with s123 b1
        for u in range(8):
            s123_unit(1, u)
            up_unit(0, u, help_dve=False)

        # ---- phase D: norm+transpose+M1 b1
        norm_m1(1, help_dve=True)

        # ---- phase E: up/out b1
        for u in range(8):
            up_unit(1, u, help_dve=True)

    nc.compile()
    return nc


def _host_prep(input, fc_weight3):
    inp = np.ascontiguousarray(input, dtype=np.float16)
    w0 = np.ascontiguousarray(fc_weight3[:, :, 0], dtype=np.float32)
    # w0: (N,16) with n=(b,gy,gx), k=(py,px)
    w0 = w0.reshape(B, 4, 32, G, PP, PP)          # b yt gy_l gx py px
    # per-partition row p=(gy_l,py), free=(yt, gx, px): contiguous per
    # batch; each weight duplicated x2 (dense inner pair for DVE 2x mode)
    w_arr = w0.transpose(0, 2, 4, 1, 3, 5).reshape(B, 128, 2048)
    w_arr = np.ascontiguousarray(
        np.repeat(w_arr, 2, axis=2).astype(np.float16))

    A = _interp_matrix_np(G, S)                   # (512,128)
    at = np.ascontiguousarray(A.T.astype(np.float16))  # (128,512)

    sel = np.zeros((128, 512), dtype=np.float16)
    p = np.arange(128)
    for j in range(4):
        sel[p, j * 128 + 32 * j + p // 4] = 1.0

    ident = np.eye(128, dtype=np.float16)
    return inp, w_arr, at, sel, ident


def kernel(input, fc_weight3):
    from concourse.bass_utils import run_bass_kernel_spmd

    if "nc" not in _CACHE:
        _CACHE["nc"] = _build_program()
    nc = _CACHE["nc"]

    inp, w_arr, at, sel, ident = _host_prep(input, fc_weight3)

    in_maps = []
    for c in range(NCORES):
        in_maps.append({
            "x": inp[c * BPC:(c + 1) * BPC],
            "w": w_arr[c * BPC:(c + 1) * BPC],
            "at": at,
            "sel": sel,
            "ident": ident,
        })
    res = run_bass_kernel_spmd(nc, in_maps, list(range(NCORES)))
    out = np.concatenate([r["y"] for r in res.results], axis=0)
    return out.reshape(-1, T).astype(np.float32)


# revision 22
# speedup vs baseline: 1.0939x; 1.0939x over previous
"""Trainium2 Bass kernel for nn_Fine_Change_Moment3.

Math (from the reference):
  - input (16,512,512,16) [b,y,x,t]; fc_weight3 (262144,16,6) per-patch 16x6.
  - Only channel 0 of the CAM survives (cam[:, 0]), so only
    fc_weight3[:, :, 0] matters (host-sliced).
  - Per 4x4 patch n=(b,gy,gx): cam0[t] = sum_k patch[k,t] * w[n,k]
  - v = (cam0 - min_t) / max_t(cam0 - min_t)
  - top[b,t] = v arranged (gy,gx); up = A @ top @ A^T with A the 128->512
    bilinear (align_corners) interp matrix; output (b*512*512, 16) f32.

Distribution: data-parallel over batch, 2 batches per core, 8 cores.

fp16 end-to-end on device: the host casts input and weights to fp16 and
casts the fp16 output back to f32, halving HBM traffic (the DMA roofline)
and doubling DVE/PE throughput (2e-2 tolerance; fp16 leaves ~1e-2 margin).

Engine queues are FIFO, so program order is the schedule. The two batches
are software-pipelined explicitly:

  A: load+weight+reduce (s123) b0          DVE-paced, in_b0 streams
  B: norm+transpose+M1 b0; prefetch in_b1
  C: up/out b0 INTERLEAVED with s123 b1    ACT+PE run b0's tail while
                                           DVE consumes b1's input;
                                           out_b0 + in_b1 share DMA
  D: norm+transpose+M1 b1
  E: up/out b1                             copies split DVE/ACT (nothing
                                           queues behind them)

Stage detail:
  s123: DVE multiply by per-patch weights (weights host-duplicated x2 so
        the t-broadcast keeps a dense step-1 inner pair -> 2x DVE mode),
        DVE pairwise add tree over px, PE 0/1-selection matmul reduces py
        and regroups gy -> cam[gy][(gx,t)] accumulated fp32 in PSUM.
  norm: DVE min/max over t; min/recip pair-duplicated (gpsimd) so the
        broadcast subtract/multiply stay in 2x mode.
  M1:   PE per-t transposes then M1_t = topT_t^T @ A^T, ACT interleaves
        pairs into m1i[gy][(sx,t)].
  up:   PE (A^T chunk)^T @ m1i chunk -> PSUM, copies cast to fp16 staging,
        contiguous DMA out.
"""

import numpy as np

B, S, T, PP = 16, 512, 16, 4
G = S // PP          # 128 patch grid
NCORES = 8
BPC = B // NCORES    # 2 batches per core

_CACHE = {}


def _interp_matrix_np(n_in, n_out):
    # mirrors the reference's align_corners=True bilinear matrix
    coords = np.arange(n_out, dtype=np.float32) * ((n_in - 1) / (n_out - 1))
    i0 = np.clip(np.floor(coords).astype(np.int64), 0, n_in - 2)
    w = coords - i0.astype(np.float32)
    A = np.zeros((n_out, n_in), dtype=np.float32)
    rows = np.arange(n_out)
    np.add.at(A, (rows, i0), 1.0 - w)
    np.add.at(A, (rows, i0 + 1), w)
    return A  # (n_out, n_in)


def _build_program():
    from contextlib import ExitStack
    import concourse.bacc as bacc
    import concourse.tile as tile
    import concourse.mybir as mybir

    f32 = mybir.dt.float32
    f16 = mybir.dt.float16
    Alu = mybir.AluOpType
    Ax = mybir.AxisListType

    nc = bacc.Bacc("TRN2", target_bir_lowering=False, debug=False,
                   num_devices=NCORES)

    x_d = nc.dram_tensor("x", [BPC, S, S, T], f16, kind="ExternalInput")
    w_d = nc.dram_tensor("w", [BPC, 128, 4096], f16, kind="ExternalInput")
    at_d = nc.dram_tensor("at", [128, 512], f16, kind="ExternalInput")
    sel_d = nc.dram_tensor("sel", [128, 512], f16, kind="ExternalInput")
    id_d = nc.dram_tensor("ident", [128, 128], f16, kind="ExternalInput")
    y_d = nc.dram_tensor("y", [BPC, S, S, T], f16, kind="ExternalOutput")

    # input view: [b][yt][xh][y_row=128][(x256 t16)=4096]
    x_v = x_d.ap().rearrange("b (yt p) (xh xx) t -> b yt xh p (xx t)",
                             p=128, xh=2)
    # output view: [b][syc][xh][sy=128][(xx256 t16)=4096]
    y_v = y_d.ap().rearrange("b (syc sy) (xh xx) t -> b syc xh sy (xx t)",
                             syc=4, xh=2)

    with tile.TileContext(nc) as tc, ExitStack() as ctx:
        consts = ctx.enter_context(tc.tile_pool(name="consts", bufs=1))
        pin = ctx.enter_context(tc.tile_pool(name="pin", bufs=8))
        pw = ctx.enter_context(tc.tile_pool(name="pw", bufs=2))
        pp2 = ctx.enter_context(tc.tile_pool(name="pp2", bufs=4))
        pv = ctx.enter_context(tc.tile_pool(name="pv", bufs=1))
        ptop = ctx.enter_context(tc.tile_pool(name="ptop", bufs=1))
        pm1 = ctx.enter_context(tc.tile_pool(name="pm1", bufs=1))
        pst = ctx.enter_context(tc.tile_pool(name="pst", bufs=3))
        ps1 = ctx.enter_context(tc.tile_pool(name="ps1", bufs=2))
        # PSUM: cam+transpose share a 4-bank pool; m1/up share a
        # 2x 2-bank pool -> static total 8 banks.
        ppsc = ctx.enter_context(tc.tile_pool(name="ppsc", bufs=1,
                                              space="PSUM"))
        ppsmu = ctx.enter_context(tc.tile_pool(name="ppsmu", bufs=2,
                                               space="PSUM"))

        at_sb = consts.tile([128, 512], f16)
        sel_sb = consts.tile([128, 512], f16)
        id_sb = consts.tile([128, 128], f16)

        state = {}

        def s123_open(b):
            # weight prefetch only; cam PSUM is allocated lazily at its
            # first matmul so the ppsc ring order stays cam(b)/tp(b)
            state[b] = {
                "cam_ps": None,
                "w_sb": pw.tile([128, 4096], f16, tag="w", name="w_sb"),
                "p2": None,
            }
            nc.sync.dma_start(state[b]["w_sb"][:], w_d.ap()[b])

        def s123_unit(b, u):
            # one (yt, xh) input unit: DMA + multiply + add tree (+ cam
            # matmuls when the yt pair completes)
            yt, xh = u // 2, u % 2
            st = state[b]
            w_sb = st["w_sb"]
            if xh == 0:
                st["p2"] = pp2.tile([128, 2048], f16, tag="p2", name="p2")
            p2 = st["p2"]
            it = pin.tile([128, 4096], f16, tag="in")
            nc.sync.dma_start(it[:], x_v[b, yt, xh])
            itv = it[:].rearrange("p (x tp two) -> p x tp two",
                                  tp=T // 2, two=2)
            wv = (w_sb[:, yt * 1024 + xh * 512:yt * 1024 + (xh + 1) * 512]
                  .rearrange("p (x two) -> p x two", two=2)
                  .unsqueeze(2)
                  .broadcast_to([128, 256, T // 2, 2]))
            nc.vector.tensor_tensor(itv, itv, wv, op=Alu.mult)
            pr = it[:].rearrange("p (gx pxp px2 t) -> p gx pxp px2 t",
                                 pxp=2, px2=2, t=T)
            s1 = ps1.tile([128, 2048], f16, tag="s1")
            s1v = s1[:].rearrange("p (gx pxp t) -> p gx pxp t",
                                  pxp=2, t=T)
            nc.vector.tensor_tensor(s1v, pr[:, :, :, 0, :],
                                    pr[:, :, :, 1, :], op=Alu.add)
            rout = (p2[:, xh * 1024:(xh + 1) * 1024]
                    .rearrange("p (gx t) -> p gx t", t=T))
            nc.vector.tensor_tensor(rout, s1v[:, :, 0, :],
                                    s1v[:, :, 1, :], op=Alu.add)
            if xh == 1:
                if st["cam_ps"] is None:
                    st["cam_ps"] = ppsc.tile([128, 2048], f32, tag="cam",
                                         name="cam_ps")
                cam_ps = st["cam_ps"]
                for fc in range(4):
                    nc.tensor.matmul(
                        cam_ps[:, fc * 512:(fc + 1) * 512],
                        lhsT=sel_sb[:, yt * 128:(yt + 1) * 128],
                        rhs=p2[:, fc * 512:(fc + 1) * 512],
                        start=(yt == 0), stop=(yt == 3),
                    )

        def norm_m1(b, help_dve):
            st = state[b]
            cam_ps = st["cam_ps"]

            v = pv.tile([128, 2048], f16, tag="v")
            mn = pv.tile([128, 128], f16, tag="mn")
            mnp = pv.tile([128, 256], f16, tag="mnp")
            mx = pv.tile([128, 128], f32, tag="mx")
            rx = pv.tile([128, 128], f32, tag="rx")
            rxp = pv.tile([128, 256], f16, tag="rxp")
            cam3 = cam_ps[:].rearrange("p (gx t) -> p gx t", t=T)
            cam4 = cam_ps[:].rearrange("p (gx tp two) -> p gx tp two",
                                       tp=T // 2, two=2)
            v3 = v[:].rearrange("p (gx t) -> p gx t", t=T)
            v4 = v[:].rearrange("p (gx tp two) -> p gx tp two",
                                tp=T // 2, two=2)
            nc.vector.tensor_reduce(mn[:], cam3, axis=Ax.X, op=Alu.min)
            nc.gpsimd.tensor_copy(
                mnp[:].rearrange("p (gx two) -> p gx two", two=2),
                mn[:].unsqueeze(2).broadcast_to([128, 128, 2]))
            mnb = (mnp[:].rearrange("p (gx two) -> p gx two", two=2)
                   .unsqueeze(2).broadcast_to([128, 128, T // 2, 2]))
            nc.vector.tensor_tensor(v4, cam4, mnb, op=Alu.subtract)
            nc.vector.tensor_reduce(mx[:], v3, axis=Ax.X, op=Alu.max)
            nc.vector.reciprocal(rx[:], mx[:])
            nc.gpsimd.tensor_copy(
                rxp[:].rearrange("p (gx two) -> p gx two", two=2),
                rx[:].unsqueeze(2).broadcast_to([128, 128, 2]))
            rxb = (rxp[:].rearrange("p (gx two) -> p gx two", two=2)
                   .unsqueeze(2).broadcast_to([128, 128, T // 2, 2]))
            nc.vector.tensor_tensor(v4, v4, rxb, op=Alu.mult)

            # per-t 128x128 transposes -> topT[gx][(t,gy)]
            topT = ptop.tile([128, 2048], f16, tag="top")
            vt = v[:].rearrange("p (gx t) -> p t gx", t=T)
            tp_ps = ppsc.tile([128, 2048], f16, tag="cam")
            for t in range(T):
                nc.tensor.transpose(tp_ps[:, t * 128:(t + 1) * 128],
                                    vt[:, t, :], id_sb[:])
            nc.scalar.copy(topT[:], tp_ps[:])

            # M1 per t -> m1i[gy][(sx,t)] fp16, pair-interleaved
            m1i = pm1.tile([128, 8192], f16, tag="m1i")
            m1iv = m1i[:].rearrange("p (sx t) -> p sx t", t=T)
            for tq in range(8):
                m1_ps = ppsmu.tile([128, 1024], f32, tag="mu")
                for tl in range(2):
                    t = tq * 2 + tl
                    nc.tensor.matmul(
                        m1_ps[:, tl * 512:(tl + 1) * 512],
                        lhsT=topT[:, t * 128:(t + 1) * 128],
                        rhs=at_sb[:],
                        start=True, stop=True,
                    )
                csrc = m1_ps[:].rearrange("p (tl sx) -> p sx tl", tl=2)
                cdst = m1iv[:, :, tq * 2:(tq + 1) * 2]
                if help_dve and tq % 2 == 1:
                    nc.vector.tensor_copy(cdst, csrc)
                else:
                    nc.scalar.copy(cdst, csrc)
            st["m1i"] = m1i

        def up_unit(b, u, help_dve):
            # one (syc, xh) output unit: 4x (2 matmuls + copy), DMA out
            syc, xh = u // 2, u % 2
            m1i = state[b]["m1i"]
            stg = pst.tile([128, 4096], f16, tag="stg")
            for sxg in range(4):
                up_ps = ppsmu.tile([128, 1024], f32, tag="mu")
                for sxl in range(2):
                    sxblk = (xh * 4 + sxg) * 2 + sxl
                    nc.tensor.matmul(
                        up_ps[:, sxl * 512:(sxl + 1) * 512],
                        lhsT=at_sb[:, syc * 128:(syc + 1) * 128],
                        rhs=m1i[:, sxblk * 512:(sxblk + 1) * 512],
                        start=True, stop=True,
                    )
                dst = stg[:, sxg * 1024:(sxg + 1) * 1024]
                if (help_dve and sxg % 2 == 1) or (not help_dve and sxg == 3):
                    nc.vector.tensor_copy(dst, up_ps[:])
                else:
                    nc.scalar.copy(dst, up_ps[:])
            nc.sync.dma_start(y_v[b, syc, xh], stg[:])

        # ---- phase A: s123(b0); consts ride behind the first input DMAs
        s123_open(0)
        s123_unit(0, 0)
        nc.sync.dma_start(at_sb[:], at_d.ap())
        nc.sync.dma_start(sel_sb[:], sel_d.ap())
        nc.sync.dma_start(id_sb[:], id_d.ap())
        for u in range(1, 8):
            s123_unit(0, u)

        # ---- phase B: norm+transpose+M1 b0; prefetch b1's first inputs
        s123_open(1)
        norm_m1(0, help_dve=False)

        # ---- phase C: up/out b0 interleaved with s123 b1
        for u in range(8):
            s123_unit(1, u)
            up_unit(0, u, help_dve=False)

        # ---- phase D: norm+transpose+M1 b1
        norm_m1(1, help_dve=True)

        # ---- phase E: up/out b1
        for u in range(8):
            up_unit(1, u, help_dve=True)

    nc.compile()
    return nc


def _host_prep(input, fc_weight3):
    inp = np.ascontiguousarray(input, dtype=np.float16)
    w0 = np.ascontiguousarray(fc_weight3[:, :, 0], dtype=np.float32)
    # w0: (N,16) with n=(b,gy,gx), k=(py,px)
    w0 = w0.reshape(B, 4, 32, G, PP, PP)          # b yt gy_l gx py px
    # per-partition row p=(gy_l,py), free=(yt, gx, px): contiguous per
    # batch; each weight duplicated x2 (dense inner pair for DVE 2x mode)
    w_arr = w0.transpose(0, 2, 4, 1, 3, 5).reshape(B, 128, 2048)
    w_arr = np.ascontiguousarray(
        np.repeat(w_arr, 2, axis=2).astype(np.float16))

    A = _interp_matrix_np(G, S)                   # (512,128)
    at = np.ascontiguousarray(A.T.astype(np.float16))  # (128,512)

    sel = np.zeros((128, 512), dtype=np.float16)
    p = np.arange(128)
    for j in range(4):
        sel[p, j * 128 + 32 * j + p // 4] = 1.0

    ident = np.eye(128, dtype=np.float16)
    return inp, w_arr, at, sel, ident


def kernel(input, fc_weight3):
    from concourse.bass_utils import run_bass_kernel_spmd

    if "nc" not in _CACHE:
        _CACHE["nc"] = _build_program()
    nc = _CACHE["nc"]

    inp, w_arr, at, sel, ident = _host_prep(input, fc_weight3)

    in_maps = []
    for c in range(NCORES):
        in_maps.append({
            "x": inp[c * BPC:(c + 1) * BPC],
            "w": w_arr[c * BPC:(c + 1) * BPC],
            "at": at,
            "sel": sel,
            "ident": ident,
        })
    res = run_bass_kernel_spmd(nc, in_maps, list(range(NCORES)))
    out = np.concatenate([r["y"] for r in res.results], axis=0)
    return out.reshape(-1, T).astype(np.float32)
